# revision 28
# baseline (speedup 1.0000x reference)
"""Bass/Trainium2 kernel for BERT-style masked attention (B=1, S=4096, HID=1024, H=16).

Strategy: tensor-parallel over heads across 8 NeuronCores (2 heads/core).
Each core computes q/k/v projections for its 128 output columns from the
full (host-pretransposed) hidden states, runs masked softmax attention for
its 2 heads fully on-chip (flash-style, scores never hit DRAM), and writes
its [4096, 128] slice of the context. Host concatenates slices.

The key mask is key-only (same for every query/head), so masked key
positions are compacted away host-side: k/v projections and the attention
inner loop run only over the ~(S/2) surviving key positions.

Overlap structure: the k/v projection + v-transpose pipeline is interleaved
per 512-block with query-tile 0's attention, and each later query tile's
projection is drip-fed (one matmul per key chunk) through the preceding
tile's attention loop, so the PE and ACT engines stay dense end-to-end.
"""

import numpy as np
from contextlib import ExitStack

import concourse.bass as bass
import concourse.tile as tile
from concourse import bacc, mybir
from concourse.bass_utils import run_bass_kernel_spmd
from concourse.masks import make_identity

f32 = mybir.dt.float32
f32r = mybir.dt.float32r
bf16 = mybir.dt.bfloat16
AF = mybir.ActivationFunctionType

S = 4096
HID = 1024
D2 = 128          # per-core output columns (2 heads x 64)
NCH = HID // 128  # 8 hid chunks
NQT = S // 512    # 8 query tiles
SCALE = 64 ** -0.5
NEG = -1e30


def _emit(nc, tc, aps, nkb, nkca):
    """nkb: # 512-wide key blocks for k/v projections (SKP = 512*nkb).
    nkca: # 128-wide key chunks the attention loop visits (<= 4*nkb)."""
    XT, XTKV, WQ, WK, WV, BQ, BK, BV, MB, ONE64, OUT = aps
    skp = 512 * nkb
    with ExitStack() as top:
        const = top.enter_context(tc.tile_pool(name="const", bufs=1))
        big = top.enter_context(tc.tile_pool(name="big", bufs=1))

        ident = const.tile([128, 128], f32)
        make_identity(nc, ident)

        wq = const.tile([128, NCH, 128], f32r)
        wk = const.tile([128, NCH, 128], f32r)
        wv = const.tile([128, NCH, 128], f32r)
        nc.sync.dma_start(out=wk, in_=WK.rearrange("(c p) d -> p c d", p=128).bitcast(f32r))
        nc.scalar.dma_start(out=wq, in_=WQ.rearrange("(c p) d -> p c d", p=128).bitcast(f32r))
        nc.scalar.dma_start(out=wv, in_=WV.rearrange("(c p) d -> p c d", p=128).bitcast(f32r))

        bq = const.tile([128, 1], f32)
        bk = const.tile([128, 1], f32)
        bv = const.tile([128, 1], f32)
        nc.gpsimd.dma_start(out=bq, in_=BQ.unsqueeze(1))
        nc.gpsimd.dma_start(out=bk, in_=BK.unsqueeze(1))
        nc.gpsimd.dma_start(out=bv, in_=BV.unsqueeze(1))

        mb = const.tile([128, nkca], f32)
        nc.gpsimd.dma_start(out=mb, in_=MB)

        qT = big.tile([128, S], f32r)     # [d2, s] queries (both heads stacked)
        kT = big.tile([128, skp], f32r)   # [d2, sk] keys (compacted)
        vT = big.tile([128, skp], f32)    # [d2, sk] values (pre-transpose)
        v1 = big.tile([128, 2, nkca, 65], f32r)  # [k, head, chunk, d|1]
        ones_c = const.tile([128, 1], f32)
        nc.vector.memset(ones_c, 1.0)
        ones_r64 = const.tile([65, 64], f32r)
        nc.gpsimd.dma_start(out=ones_r64[64:65, :], in_=ONE64.unsqueeze(0).bitcast(f32r))
        nc.vector.tensor_copy(v1[:, 0, :, 64:65], ones_c.to_broadcast((128, nkca, 1)))
        nc.vector.tensor_copy(v1[:, 1, :, 64:65], ones_c.to_broadcast((128, nkca, 1)))

        h0 = slice(0, 64)
        h1 = slice(64, 128)
        # can the next q tile's projection be drip-fed through the kc loop?
        drip = nkca >= NCH + 2
        d0 = min(3, nkca - NCH - 1)  # drip start; finish early

        with tc.tile_pool(name="xwkp", bufs=3) as xwkp, \
             tc.tile_pool(name="xwp", bufs=3) as xwp, \
             tc.tile_pool(name="pkv", bufs=1, space="PSUM") as pkv, \
             tc.tile_pool(name="ppq", bufs=1, space="PSUM") as ppq, \
             tc.tile_pool(name="pss", bufs=2, space="PSUM") as pss, \
             tc.tile_pool(name="psc", bufs=1, space="PSUM") as psc, \
             tc.tile_pool(name="ep", bufs=4) as ep, \
             tc.tile_pool(name="op", bufs=4) as op, \
             tc.tile_pool(name="cp", bufs=4) as cp, \
             tc.tile_pool(name="lp", bufs=2) as lp:

            qstate = {}

            def qproj_dma(qt):
                qsl = slice(qt * 512, (qt + 1) * 512)
                xw = xwp.tile([128, NCH, 512], f32r, tag="xw", name=f"xw{qt}")
                for c in range(NCH):
                    nc.sync.dma_start(
                        out=xw[:, c, :],
                        in_=XT[c * 128:(c + 1) * 128, qsl].bitcast(f32r))
                pq = ppq.tile([128, 512], f32, tag="pqq", name=f"pq{qt}")
                qstate[qt] = (xw, pq)

            def qproj_mm(qt, c):
                xw, pq = qstate[qt]
                nc.tensor.matmul(pq, wq[:, c, :], xw[:, c, :],
                                 start=(c == 0), stop=(c == NCH - 1),
                                 skip_group_check=True)
                if c == NCH - 1:
                    qsl = slice(qt * 512, (qt + 1) * 512)
                    nc.vector.tensor_scalar_add(qT[:, qsl], pq, bq)

            def k_block(kb, xw=None):
                sl = slice(kb * 512, (kb + 1) * 512)
                if xw is None:
                    xw = xwkp.tile([128, NCH, 512], f32r, tag="xwk",
                                   name=f"xwk{kb}")
                    for c in range(NCH):
                        nc.sync.dma_start(
                            out=xw[:, c, :],
                            in_=XTKV[c * 128:(c + 1) * 128, sl].bitcast(f32r))
                pk = pkv.tile([128, 512], f32, tag="pkv", name=f"pk{kb}")
                for c in range(NCH):
                    nc.tensor.matmul(pk, wk[:, c, :], xw[:, c, :],
                                     start=(c == 0), stop=(c == NCH - 1))
                nc.vector.tensor_scalar_add(kT[:, sl], pk, bk)
                return xw

            def v_block(kb, xw):
                sl = slice(kb * 512, (kb + 1) * 512)
                pv = pkv.tile([128, 512], f32, tag="pkv", name=f"pv{kb}")
                for c in range(NCH):
                    nc.tensor.matmul(pv, wv[:, c, :], xw[:, c, :],
                                     start=(c == 0), stop=(c == NCH - 1))
                nc.vector.tensor_scalar_add(vT[:, sl], pv, bv)

            def vt_chunk(kc):
                pt = pss.tile([128, 2, 512], f32, tag="ss", name=f"vt{kc}")
                nc.tensor.transpose(pt[:, 0, 0:128],
                                    vT[:, kc * 128:(kc + 1) * 128], ident)
                nc.vector.tensor_copy(v1[:, 0, kc, 0:64], pt[:, 0, 0:64])
                nc.vector.tensor_copy(v1[:, 1, kc, 0:64], pt[:, 0, 64:128])

            att = {}

            def att_begin(qt):
                if not drip or qt == 0:
                    qproj_dma(qt)
                    for c in range(NCH):
                        qproj_mm(qt, c)
                pc0 = psc.tile([65, 512], f32, tag="pc0", name=f"pc0_{qt}")
                pc1 = psc.tile([65, 512], f32, tag="pc1", name=f"pc1_{qt}")
                att[qt] = [pc0, pc1, None]

            def att_chunk(qt, kc):
                st = att[qt]
                qsl = slice(qt * 512, (qt + 1) * 512)
                ksl = slice(kc * 128, (kc + 1) * 128)
                ss = pss.tile([128, 2, 512], f32, tag="ss", name=f"ss{qt}_{kc}")
                nc.tensor.matmul(ss[:, 0, :], kT[h0, ksl], qT[h0, qsl],
                                 start=True, stop=True)
                nc.tensor.matmul(ss[:, 1, :], kT[h1, ksl], qT[h1, qsl],
                                 start=True, stop=True)
                et = ep.tile([128, 2, 512], f32r, tag="et", name=f"et{qt}_{kc}")
                bias = mb[:, nkca - 1:nkca] if kc == nkca - 1 else 0.0
                nc.scalar.activation(et, ss, AF.Exp, bias=bias, scale=SCALE)
                if st[2] is not None:
                    pkc, pet = st[2]
                    nc.tensor.matmul(st[0], v1[:, 0, pkc, :], pet[:, 0, :],
                                     start=(pkc == 0), stop=False)
                    nc.tensor.matmul(st[1], v1[:, 1, pkc, :], pet[:, 1, :],
                                     start=(pkc == 0), stop=False)
                st[2] = (kc, et)
                if drip and qt + 1 < NQT:
                    if kc == d0:
                        qproj_dma(qt + 1)
                    elif d0 + 1 <= kc <= d0 + NCH:
                        qproj_mm(qt + 1, kc - d0 - 1)

            def att_flush(qt):
                pc0, pc1, prev = att[qt]
                pkc, pet = prev
                nc.tensor.matmul(pc0, v1[:, 0, pkc, :], pet[:, 0, :],
                                 start=(pkc == 0), stop=True)
                nc.tensor.matmul(pc1, v1[:, 1, pkc, :], pet[:, 1, :],
                                 start=(pkc == 0), stop=True)
                att_drain(qt)

            def att_drain(qt):
                # free the PSUM accumulators right away: l (row 64) to SBUF
                # + unnormalized ctx [64,512] to SBUF, per head.
                res = []
                for h in range(2):
                    pc = att[qt][h]
                    lr = lp.tile([65, 512], f32r, tag="linv",
                                 name=f"l{qt}_{h}")
                    nc.vector.tensor_copy(lr[64:65, :], pc[64:65, :])
                    ct = cp.tile([64, 512], f32, tag="ct",
                                 name=f"ct{qt}_{h}")
                    nc.vector.tensor_copy(ct, pc[0:64, :])
                    res.append((lr, ct))
                att[qt].append(res)

            def att_norm(qt, h):
                # broadcast l over the 64 d-partitions via a K=1 matmul,
                # approx-reciprocal on all lanes, multiply, DMA out (output
                # stays transposed [d, q]; host untransposes).
                lr, ct = att[qt][3][h]
                lbc = pkv.tile([128, 512], f32, tag="pkv",
                               name=f"lbc{qt}_{h}")
                nc.tensor.matmul(lbc[0:64, :], ones_r64[64:65, :],
                                 lr[64:65, :], start=True, stop=True)
                linv = op.tile([64, 512], f32, tag="lbs")
                nc.vector.reciprocal_approx_fast(out=linv, in_=lbc[0:64, :])
                ob = op.tile([64, 512], f32, tag="ob")
                nc.vector.tensor_mul(ob, linv, ct)
                nc.sync.dma_start(
                    out=OUT[h * 64:(h + 1) * 64, qt * 512:(qt + 1) * 512],
                    in_=ob)

            # ---- qt 0: interleaved with k/v projection blocks.
            # Critical-path DMAs first: qt0's x chunks and kv block 0's x
            # chunks alternate on the queue so both projection chains start
            # as early as possible.
            qsl0 = slice(0, 512)
            xw0 = xwp.tile([128, NCH, 512], f32r, tag="xw", name="xw0")
            xwk0 = xwkp.tile([128, NCH, 512], f32r, tag="xwk", name="xwk0")
            for c in range(NCH):
                nc.sync.dma_start(
                    out=xwk0[:, c, :],
                    in_=XTKV[c * 128:(c + 1) * 128, 0:512].bitcast(f32r))
                nc.sync.dma_start(
                    out=xw0[:, c, :],
                    in_=XT[c * 128:(c + 1) * 128, qsl0].bitcast(f32r))
            pq0 = ppq.tile([128, 512], f32, tag="pqq", name="pq0")
            qstate[0] = (xw0, pq0)
            for c in range(NCH):
                qproj_mm(0, c)
            pc0_t = psc.tile([65, 512], f32, tag="pc0", name="pc0_0")
            pc1_t = psc.tile([65, 512], f32, tag="pc1", name="pc1_0")
            att[0] = [pc0_t, pc1_t, None]
            for kb in range(nkb):
                xw = k_block(kb, xw=xwk0 if kb == 0 else None)
                if kb * 4 < nkca:
                    att_chunk(0, kb * 4)  # ctx inside uses v1[kc-1] (ready)
                v_block(kb, xw)
                for j in range(4):
                    kc = kb * 4 + j
                    if kc >= nkca:
                        break
                    vt_chunk(kc)
                    if j > 0:
                        att_chunk(0, kc)
            att_flush(0)
            # ---- remaining q tiles; previous tile's normalization is
            # deferred into the early chunks of the current tile
            for qt in range(1, NQT):
                att_begin(qt)
                for kc in range(nkca):
                    if kc in (1, 2):
                        att_norm(qt - 1, kc - 1)
                    att_chunk(qt, kc)
                for h in range(min(2, max(0, nkca - 1)), 2):
                    att_norm(qt - 1, h)  # leftovers when nkca is tiny
                att_flush(qt)
            for h in range(2):
                att_norm(NQT - 1, h)


_NC = {}


def _build(nkb, nkca):
    key = (nkb, nkca)
    if key in _NC:
        return _NC[key]
    nc = bacc.Bacc("TRN2", target_bir_lowering=False, debug=False)
    skp = 512 * nkb
    XT = nc.dram_tensor("XT", [HID, S], f32, kind="ExternalInput").ap()
    XTKV = nc.dram_tensor("XTKV", [HID, skp], f32, kind="ExternalInput").ap()
    WQ = nc.dram_tensor("WQ", [HID, D2], f32, kind="ExternalInput").ap()
    WK = nc.dram_tensor("WK", [HID, D2], f32, kind="ExternalInput").ap()
    WV = nc.dram_tensor("WV", [HID, D2], f32, kind="ExternalInput").ap()
    BQ = nc.dram_tensor("BQ", [D2], f32, kind="ExternalInput").ap()
    BK = nc.dram_tensor("BK", [D2], f32, kind="ExternalInput").ap()
    BV = nc.dram_tensor("BV", [D2], f32, kind="ExternalInput").ap()
    MB = nc.dram_tensor("MB", [128, nkca], f32, kind="ExternalInput").ap()
    ONE64 = nc.dram_tensor("ONE64", [64], f32, kind="ExternalInput").ap()
    OUT = nc.dram_tensor("OUT", [D2, S], f32, kind="ExternalOutput").ap()
    with tile.TileContext(nc) as tc:
        _emit(nc, tc, (XT, XTKV, WQ, WK, WV, BQ, BK, BV, MB, ONE64, OUT), nkb, nkca)
    nc.compile()
    _NC[key] = nc
    return nc


def make_in_maps(hidden_states, attention_mask, Wq, bq, Wk, bk, Wv, bv):
    x = np.asarray(hidden_states, dtype=np.float32).reshape(S, HID)
    xT = np.ascontiguousarray(x.T)
    mask = np.asarray(attention_mask).reshape(S).astype(bool)
    idx = np.nonzero(mask)[0]
    m = len(idx)
    nkca = max(1, (m + 127) // 128)
    nkb = max(1, (nkca * 128 + 511) // 512)
    skp = nkb * 512
    # pad with position 0 (values are finite; pad slots masked to -inf below)
    idx_p = np.zeros(skp, np.int64)
    idx_p[:m] = idx
    xTkv = np.ascontiguousarray(xT[:, idx_p])
    mbias = np.full(nkca * 128, np.float32(NEG), np.float32)
    mbias[:m] = 0.0
    MBn = np.ascontiguousarray(mbias.reshape(nkca, 128).T)
    Wq = np.asarray(Wq, np.float32)
    Wk = np.asarray(Wk, np.float32)
    Wv = np.asarray(Wv, np.float32)
    bq = np.asarray(bq, np.float32)
    bk = np.asarray(bk, np.float32)
    bv = np.asarray(bv, np.float32)
    in_maps = []
    for c in range(8):
        sl = slice(D2 * c, D2 * (c + 1))
        in_maps.append({
            "XT": xT, "XTKV": xTkv, "MB": MBn,
            "WQ": np.ascontiguousarray(Wq[:, sl]),
            "WK": np.ascontiguousarray(Wk[:, sl]),
            "WV": np.ascontiguousarray(Wv[:, sl]),
            "BQ": np.ascontiguousarray(bq[sl]),
            "BK": np.ascontiguousarray(bk[sl]),
            "BV": np.ascontiguousarray(bv[sl]),
            "ONE64": np.ones(64, np.float32),
        })
    return in_maps, nkb, nkca


def kernel(hidden_states, attention_mask, Wq, bq, Wk, bk, Wv, bv):
    in_maps, nkb, nkca = make_in_maps(
        hidden_states, attention_mask, Wq, bq, Wk, bk, Wv, bv)
    nc = _build(nkb, nkca)
    res = run_bass_kernel_spmd(nc, in_maps, list(range(8)))
    outT = np.concatenate([res.results[c]["OUT"] for c in range(8)], axis=0)
    return (np.ascontiguousarray(outT.T).reshape(1, S, HID),)


# revision 29
# speedup vs baseline: 1.0020x; 1.0020x over previous
"""Bass/Trainium2 kernel for BERT-style masked attention (B=1, S=4096, HID=1024, H=16).

Strategy: tensor-parallel over heads across 8 NeuronCores (2 heads/core).
Each core computes q/k/v projections for its 128 output columns from the
full (host-pretransposed) hidden states, runs masked softmax attention for
its 2 heads fully on-chip (flash-style, scores never hit DRAM), and writes
its [4096, 128] slice of the context. Host concatenates slices.

The key mask is key-only (same for every query/head), so masked key
positions are compacted away host-side: k/v projections and the attention
inner loop run only over the ~(S/2) surviving key positions.

Overlap structure: the k/v projection + v-transpose pipeline is interleaved
per 512-block with query-tile 0's attention, and each later query tile's
projection is drip-fed (one matmul per key chunk) through the preceding
tile's attention loop, so the PE and ACT engines stay dense end-to-end.
"""

import numpy as np
from contextlib import ExitStack

import concourse.bass as bass
import concourse.tile as tile
from concourse import bacc, mybir
from concourse.bass_utils import run_bass_kernel_spmd
from concourse.masks import make_identity

f32 = mybir.dt.float32
f32r = mybir.dt.float32r
bf16 = mybir.dt.bfloat16
AF = mybir.ActivationFunctionType

S = 4096
HID = 1024
D2 = 128          # per-core output columns (2 heads x 64)
NCH = HID // 128  # 8 hid chunks
NQT = S // 512    # 8 query tiles
SCALE = 64 ** -0.5
NEG = -1e30


def _emit(nc, tc, aps, nkb, nkca):
    """nkb: # 512-wide key blocks for k/v projections (SKP = 512*nkb).
    nkca: # 128-wide key chunks the attention loop visits (<= 4*nkb)."""
    XT, XTKV, WQ, WK, WV, BQ, BK, BV, MB, ONE64, OUT = aps
    skp = 512 * nkb
    with ExitStack() as top:
        const = top.enter_context(tc.tile_pool(name="const", bufs=1))
        big = top.enter_context(tc.tile_pool(name="big", bufs=1))

        ident = const.tile([128, 128], f32)
        make_identity(nc, ident)

        wq = const.tile([128, NCH, 128], f32r)
        wk = const.tile([128, NCH, 128], f32r)
        wv = const.tile([128, NCH, 128], f32r)
        nc.sync.dma_start(out=wk, in_=WK.rearrange("(c p) d -> p c d", p=128).bitcast(f32r))
        nc.scalar.dma_start(out=wv, in_=WV.rearrange("(c p) d -> p c d", p=128).bitcast(f32r))
        nc.scalar.dma_start(out=wq, in_=WQ.rearrange("(c p) d -> p c d", p=128).bitcast(f32r))

        bq = const.tile([128, 1], f32)
        bk = const.tile([128, 1], f32)
        bv = const.tile([128, 1], f32)
        nc.gpsimd.dma_start(out=bq, in_=BQ.unsqueeze(1))
        nc.gpsimd.dma_start(out=bk, in_=BK.unsqueeze(1))
        nc.gpsimd.dma_start(out=bv, in_=BV.unsqueeze(1))

        mb = const.tile([128, nkca], f32)
        nc.gpsimd.dma_start(out=mb, in_=MB)

        qT = big.tile([128, S], f32r)     # [d2, s] queries (both heads stacked)
        kT = big.tile([128, skp], f32r)   # [d2, sk] keys (compacted)
        vT = big.tile([128, skp], f32)    # [d2, sk] values (pre-transpose)
        v1 = big.tile([128, 2, nkca, 65], f32r)  # [k, head, chunk, d|1]
        ones_c = const.tile([128, 1], f32)
        nc.vector.memset(ones_c, 1.0)
        ones_r64 = const.tile([65, 64], f32r)
        nc.gpsimd.dma_start(out=ones_r64[64:65, :], in_=ONE64.unsqueeze(0).bitcast(f32r))
        nc.vector.tensor_copy(v1[:, 0, :, 64:65], ones_c.to_broadcast((128, nkca, 1)))
        nc.vector.tensor_copy(v1[:, 1, :, 64:65], ones_c.to_broadcast((128, nkca, 1)))

        h0 = slice(0, 64)
        h1 = slice(64, 128)
        # can the next q tile's projection be drip-fed through the kc loop?
        drip = nkca >= NCH + 2
        d0 = nkca - NCH - 1  # chunk index at which the drip starts

        with tc.tile_pool(name="xwkp", bufs=3) as xwkp, \
             tc.tile_pool(name="xwp", bufs=3) as xwp, \
             tc.tile_pool(name="pkv", bufs=1, space="PSUM") as pkv, \
             tc.tile_pool(name="ppq", bufs=1, space="PSUM") as ppq, \
             tc.tile_pool(name="pss", bufs=2, space="PSUM") as pss, \
             tc.tile_pool(name="psc", bufs=1, space="PSUM") as psc, \
             tc.tile_pool(name="ep", bufs=3) as ep, \
             tc.tile_pool(name="op", bufs=4) as op, \
             tc.tile_pool(name="cp", bufs=4) as cp, \
             tc.tile_pool(name="lp", bufs=2) as lp:

            qstate = {}

            def qproj_dma(qt):
                qsl = slice(qt * 512, (qt + 1) * 512)
                xw = xwp.tile([128, NCH, 512], f32r, tag="xw", name=f"xw{qt}")
                for c in range(NCH):
                    nc.sync.dma_start(
                        out=xw[:, c, :],
                        in_=XT[c * 128:(c + 1) * 128, qsl].bitcast(f32r))
                pq = ppq.tile([128, 512], f32, tag="pqq", name=f"pq{qt}")
                qstate[qt] = (xw, pq)

            def qproj_mm(qt, c):
                xw, pq = qstate[qt]
                nc.tensor.matmul(pq, wq[:, c, :], xw[:, c, :],
                                 start=(c == 0), stop=(c == NCH - 1),
                                 skip_group_check=True)
                if c == NCH - 1:
                    qsl = slice(qt * 512, (qt + 1) * 512)
                    nc.vector.tensor_scalar_add(qT[:, qsl], pq, bq)

            def k_block(kb, xw=None):
                sl = slice(kb * 512, (kb + 1) * 512)
                if xw is None:
                    xw = xwkp.tile([128, NCH, 512], f32r, tag="xwk",
                                   name=f"xwk{kb}")
                    for c in range(NCH):
                        nc.sync.dma_start(
                            out=xw[:, c, :],
                            in_=XTKV[c * 128:(c + 1) * 128, sl].bitcast(f32r))
                pk = pkv.tile([128, 512], f32, tag="pkv", name=f"pk{kb}")
                for c in range(NCH):
                    nc.tensor.matmul(pk, wk[:, c, :], xw[:, c, :],
                                     start=(c == 0), stop=(c == NCH - 1))
                nc.vector.tensor_scalar_add(kT[:, sl], pk, bk)
                return xw

            def v_block(kb, xw):
                sl = slice(kb * 512, (kb + 1) * 512)
                pv = pkv.tile([128, 512], f32, tag="pkv", name=f"pv{kb}")
                for c in range(NCH):
                    nc.tensor.matmul(pv, wv[:, c, :], xw[:, c, :],
                                     start=(c == 0), stop=(c == NCH - 1))
                nc.vector.tensor_scalar_add(vT[:, sl], pv, bv)

            def vt_chunk(kc):
                pt = pss.tile([128, 2, 512], f32, tag="ss", name=f"vt{kc}")
                nc.tensor.transpose(pt[:, 0, 0:128],
                                    vT[:, kc * 128:(kc + 1) * 128], ident)
                nc.vector.tensor_copy(v1[:, 0, kc, 0:64], pt[:, 0, 0:64])
                nc.vector.tensor_copy(v1[:, 1, kc, 0:64], pt[:, 0, 64:128])

            att = {}

            def att_begin(qt):
                if not drip or qt == 0:
                    qproj_dma(qt)
                    for c in range(NCH):
                        qproj_mm(qt, c)
                pc0 = psc.tile([65, 512], f32, tag="pc0", name=f"pc0_{qt}")
                pc1 = psc.tile([65, 512], f32, tag="pc1", name=f"pc1_{qt}")
                att[qt] = [pc0, pc1, None]

            def att_chunk(qt, kc):
                st = att[qt]
                qsl = slice(qt * 512, (qt + 1) * 512)
                ksl = slice(kc * 128, (kc + 1) * 128)
                ss = pss.tile([128, 2, 512], f32, tag="ss", name=f"ss{qt}_{kc}")
                nc.tensor.matmul(ss[:, 0, :], kT[h0, ksl], qT[h0, qsl],
                                 start=True, stop=True)
                nc.tensor.matmul(ss[:, 1, :], kT[h1, ksl], qT[h1, qsl],
                                 start=True, stop=True)
                et = ep.tile([128, 2, 512], f32r, tag="et", name=f"et{qt}_{kc}")
                bias = mb[:, nkca - 1:nkca] if kc == nkca - 1 else 0.0
                nc.scalar.activation(et, ss, AF.Exp, bias=bias, scale=SCALE)
                if st[2] is not None:
                    pkc, pet = st[2]
                    nc.tensor.matmul(st[0], v1[:, 0, pkc, :], pet[:, 0, :],
                                     start=(pkc == 0), stop=False)
                    nc.tensor.matmul(st[1], v1[:, 1, pkc, :], pet[:, 1, :],
                                     start=(pkc == 0), stop=False)
                st[2] = (kc, et)
                if drip and qt + 1 < NQT:
                    if kc == d0:
                        qproj_dma(qt + 1)
                    elif d0 + 1 <= kc <= d0 + NCH:
                        qproj_mm(qt + 1, kc - d0 - 1)

            def att_flush(qt):
                pc0, pc1, prev = att[qt]
                pkc, pet = prev
                nc.tensor.matmul(pc0, v1[:, 0, pkc, :], pet[:, 0, :],
                                 start=(pkc == 0), stop=True)
                nc.tensor.matmul(pc1, v1[:, 1, pkc, :], pet[:, 1, :],
                                 start=(pkc == 0), stop=True)
                att_drain(qt)

            def att_drain(qt):
                # free the PSUM accumulators right away: l (row 64) to SBUF
                # + unnormalized ctx [64,512] to SBUF, per head.
                res = []
                for h in range(2):
                    pc = att[qt][h]
                    lr = lp.tile([65, 512], f32r, tag="linv",
                                 name=f"l{qt}_{h}")
                    nc.vector.tensor_copy(lr[64:65, :], pc[64:65, :])
                    ct = cp.tile([64, 512], f32, tag="ct",
                                 name=f"ct{qt}_{h}")
                    nc.vector.tensor_copy(ct, pc[0:64, :])
                    res.append((lr, ct))
                att[qt].append(res)

            def att_norm(qt, h):
                # broadcast l over the 64 d-partitions via a K=1 matmul,
                # approx-reciprocal on all lanes, multiply, DMA out (output
                # stays transposed [d, q]; host untransposes).
                lr, ct = att[qt][3][h]
                lbc = pkv.tile([128, 512], f32, tag="pkv",
                               name=f"lbc{qt}_{h}")
                nc.tensor.matmul(lbc[0:64, :], ones_r64[64:65, :],
                                 lr[64:65, :], start=True, stop=True)
                linv = op.tile([64, 512], f32, tag="lbs")
                nc.vector.reciprocal_approx_fast(out=linv, in_=lbc[0:64, :])
                ob = op.tile([64, 512], f32, tag="ob")
                nc.vector.tensor_mul(ob, linv, ct)
                nc.sync.dma_start(
                    out=OUT[h * 64:(h + 1) * 64, qt * 512:(qt + 1) * 512],
                    in_=ob)

            # ---- qt 0: interleaved with k/v projection blocks.
            # Critical-path DMAs first: qt0's x chunks and kv block 0's x
            # chunks alternate on the queue so both projection chains start
            # as early as possible.
            qsl0 = slice(0, 512)
            xw0 = xwp.tile([128, NCH, 512], f32r, tag="xw", name="xw0")
            xwk0 = xwkp.tile([128, NCH, 512], f32r, tag="xwk", name="xwk0")
            for c in range(NCH):
                nc.sync.dma_start(
                    out=xwk0[:, c, :],
                    in_=XTKV[c * 128:(c + 1) * 128, 0:512].bitcast(f32r))
                nc.sync.dma_start(
                    out=xw0[:, c, :],
                    in_=XT[c * 128:(c + 1) * 128, qsl0].bitcast(f32r))
            pq0 = ppq.tile([128, 512], f32, tag="pqq", name="pq0")
            qstate[0] = (xw0, pq0)
            for c in range(NCH):
                qproj_mm(0, c)
            pc0_t = psc.tile([65, 512], f32, tag="pc0", name="pc0_0")
            pc1_t = psc.tile([65, 512], f32, tag="pc1", name="pc1_0")
            att[0] = [pc0_t, pc1_t, None]
            for kb in range(nkb):
                xw = k_block(kb, xw=xwk0 if kb == 0 else None)
                if kb * 4 < nkca:
                    att_chunk(0, kb * 4)  # ctx inside uses v1[kc-1] (ready)
                v_block(kb, xw)
                for j in range(4):
                    kc = kb * 4 + j
                    if kc >= nkca:
                        break
                    vt_chunk(kc)
                    if j > 0:
                        att_chunk(0, kc)
            att_flush(0)
            # ---- remaining q tiles; previous tile's normalization is
            # deferred into the early chunks of the current tile
            for qt in range(1, NQT):
                att_begin(qt)
                for kc in range(nkca):
                    if kc in (1, 2):
                        att_norm(qt - 1, kc - 1)
                    att_chunk(qt, kc)
                for h in range(min(2, max(0, nkca - 1)), 2):
                    att_norm(qt - 1, h)  # leftovers when nkca is tiny
                att_flush(qt)
            for h in range(2):
                att_norm(NQT - 1, h)


_NC = {}


def _build(nkb, nkca):
    key = (nkb, nkca)
    if key in _NC:
        return _NC[key]
    nc = bacc.Bacc("TRN2", target_bir_lowering=False, debug=False)
    skp = 512 * nkb
    XT = nc.dram_tensor("XT", [HID, S], f32, kind="ExternalInput").ap()
    XTKV = nc.dram_tensor("XTKV", [HID, skp], f32, kind="ExternalInput").ap()
    WQ = nc.dram_tensor("WQ", [HID, D2], f32, kind="ExternalInput").ap()
    WK = nc.dram_tensor("WK", [HID, D2], f32, kind="ExternalInput").ap()
    WV = nc.dram_tensor("WV", [HID, D2], f32, kind="ExternalInput").ap()
    BQ = nc.dram_tensor("BQ", [D2], f32, kind="ExternalInput").ap()
    BK = nc.dram_tensor("BK", [D2], f32, kind="ExternalInput").ap()
    BV = nc.dram_tensor("BV", [D2], f32, kind="ExternalInput").ap()
    MB = nc.dram_tensor("MB", [128, nkca], f32, kind="ExternalInput").ap()
    ONE64 = nc.dram_tensor("ONE64", [64], f32, kind="ExternalInput").ap()
    OUT = nc.dram_tensor("OUT", [D2, S], f32, kind="ExternalOutput").ap()
    with tile.TileContext(nc) as tc:
        _emit(nc, tc, (XT, XTKV, WQ, WK, WV, BQ, BK, BV, MB, ONE64, OUT), nkb, nkca)
    nc.compile()
    _NC[key] = nc
    return nc


def make_in_maps(hidden_states, attention_mask, Wq, bq, Wk, bk, Wv, bv):
    x = np.asarray(hidden_states, dtype=np.float32).reshape(S, HID)
    xT = np.ascontiguousarray(x.T)
    mask = np.asarray(attention_mask).reshape(S).astype(bool)
    idx = np.nonzero(mask)[0]
    m = len(idx)
    nkca = max(1, (m + 127) // 128)
    nkb = max(1, (nkca * 128 + 511) // 512)
    skp = nkb * 512
    # pad with position 0 (values are finite; pad slots masked to -inf below)
    idx_p = np.zeros(skp, np.int64)
    idx_p[:m] = idx
    xTkv = np.ascontiguousarray(xT[:, idx_p])
    mbias = np.full(nkca * 128, np.float32(NEG), np.float32)
    mbias[:m] = 0.0
    MBn = np.ascontiguousarray(mbias.reshape(nkca, 128).T)
    Wq = np.asarray(Wq, np.float32)
    Wk = np.asarray(Wk, np.float32)
    Wv = np.asarray(Wv, np.float32)
    bq = np.asarray(bq, np.float32)
    bk = np.asarray(bk, np.float32)
    bv = np.asarray(bv, np.float32)
    in_maps = []
    for c in range(8):
        sl = slice(D2 * c, D2 * (c + 1))
        in_maps.append({
            "XT": xT, "XTKV": xTkv, "MB": MBn,
            "WQ": np.ascontiguousarray(Wq[:, sl]),
            "WK": np.ascontiguousarray(Wk[:, sl]),
            "WV": np.ascontiguousarray(Wv[:, sl]),
            "BQ": np.ascontiguousarray(bq[sl]),
            "BK": np.ascontiguousarray(bk[sl]),
            "BV": np.ascontiguousarray(bv[sl]),
            "ONE64": np.ones(64, np.float32),
        })
    return in_maps, nkb, nkca


def kernel(hidden_states, attention_mask, Wq, bq, Wk, bk, Wv, bv):
    in_maps, nkb, nkca = make_in_maps(
        hidden_states, attention_mask, Wq, bq, Wk, bk, Wv, bv)
    nc = _build(nkb, nkca)
    res = run_bass_kernel_spmd(nc, in_maps, list(range(8)))
    outT = np.concatenate([res.results[c]["OUT"] for c in range(8)], axis=0)
    return (np.ascontiguousarray(outT.T).reshape(1, S, HID),)
